# revision 13
# baseline (speedup 1.0000x reference)
"""CBOW negative-sampling loss kernel for Trainium2 (8 NeuronCores, SPMD).

Per batch element b: gather 21 rows of 50 floats (10 ctx rows from in_embed,
1 pos + 10 neg from out_embed), context sum, 11 dot products, log-sigmoids,
global mean.

v2: bulk gathers via the extended-ISA `dma_gather` (InstDMAGatherAnt)
instead of per-row indirect_dma_start.  One gather instruction fetches
thousands of random table rows (one SDMA descriptor pair per row), so the
~1.8us/op SWDGE fixed cost of the v1 kernel (2688 ops/core -> 4.8ms)
amortizes away.  HW-measured sustained cost is ~2.2-2.8ns per gathered
row at 4 SWDGE queues (the Q7 descriptor-generation floor; HBM-locality
experiments move it <10%), so the 344k rows/core run in ~720us steady
state, ~230-400us under the R=9 pairwise timing that hides pipeline
ramp under dispatch overhead.

dma_gather facts (HW-verified):
- int16 indices, gathered element must be a multiple of 256B.  VOCAB=50000
  rows don't fit int16, so the table is stored fp16 with rows padded to
  128B and viewed as [25000, 256B] blocks: block = v>>1 (max 24999), the
  wanted row is the (v&1) half.  The half-select is one in-place DVE
  copy_predicated (int8 mask, host-uploaded) per gather tile.
- single_packet=True caps a call at 64 descs/engine (1024 idxs); pass
  single_packet=False for larger calls (single_packet makes no measurable
  speed difference either way).
- Index list for a call is wrapped into 16 partitions ([16, n/16],
  idx_list[i] at [i%16, i//16]) and replicated 8x to 128 partitions
  (queue q's Q7 pair reads the replica at partitions 32q..32q+31).
- Gathered element i lands at partition i%128, free column i//128.
- 4 SWDGE queues = 4 independent Q7 desc-gen core pairs; queue count is
  the dominant throughput lever (2 queues is ~2.9x slower end to end).
- Call size barely matters (2.7-3.0 ns/row from 1k to 10k idxs/call);
  T=4 tiles/call with a 6-deep gather pool won the sweep.

Per group of T=4 tiles (512 batch elems) two gathers fetch 10*T*128 ctx
blocks from tin and 11*T*128 pos/neg blocks from tout as
[128, slots, 128f16] tiles, with ctx/out calls of group g pinned to
queues g%4 and (g+2)%4 so all four queues see equal work.  DVE then does
the predicated half-select in place, a 10->1 context tree sum, products
against the broadcast ctx vector, and a 50->1 reduce into fp32 scores
(~10.4k elems/partition/group, fully hidden under the gather pipeline).
Tail as v1: -log sig(+-s) == softplus(-+s) == Ln(1+Exp(-+0.1*s)) (the
0.1 folds the /10 context mean), Ln's accum_out yields per-partition
loss sums.  Host: loss = +(sum of partials) / B.  fp16 table keeps the
end-to-end rel err at ~1e-6 (the v1 fp8 table was ~5e-4).
"""

import sys

import numpy as np

if "/opt/trn_rl_repo" not in sys.path:
    sys.path.insert(0, "/opt/trn_rl_repo")

from concourse import bass, mybir  # noqa: E402
from concourse import bass_utils  # noqa: E402
from concourse import tile  # noqa: E402
from concourse.bacc import Bacc  # noqa: E402

VOCAB = 50000
DIM = 50
B = 131072
CTX = 10
NEG = 10

NCORES = 8
P = 128
BC = B // NCORES  # 16384
NTILES = BC // P  # 128
NBLK = VOCAB // 2  # 25000 gather blocks per table
ELEM = 128  # f16 per gather block (256B)
T = 4  # tiles per gather group
QUEUES = 4  # SWDGE queues (parallel Q7 desc-gen core pairs)
GBUFS = 6  # gather-pool double-buffering depth
SCRATCH = 16384  # SWDGE descriptor-ring carveout bytes per partition
CW = CTX * T * P // 16  # idx cols per ctx call (640)
OW = (NEG + 1) * T * P // 16  # idx cols per out call (704)
GW = CW + OW  # idx cols per group (1344)

f16 = mybir.dt.float16
f32 = mybir.dt.float32
i16 = mybir.dt.int16
i8 = mybir.dt.int8


def build_nc(ntiles: int = NTILES, repeats: int = 1, dump_scores: bool = False):
    nc = Bacc(
        None,
        target_bir_lowering=False,
        num_swdge_queues=QUEUES,
        dynamic_dma_scratch_size=SCRATCH,
    )
    one_t = nc.alloc_sbuf_tensor("const-one", [P, 1], f32)
    nc.gpsimd.memset(one_t.ap(), 1.0)
    nc.const_aps.aps[(f32, 1.0)] = one_t.ap()
    nc.all_engine_barrier()

    tin = nc.dram_tensor("tin", [NBLK, ELEM], f16, kind="ExternalInput")
    tout = nc.dram_tensor("tout", [NBLK, ELEM], f16, kind="ExternalInput")
    ngroups = ntiles // T
    assert ngroups * T == ntiles
    idx = nc.dram_tensor("idx", [P, ngroups * GW], i16, kind="ExternalInput")
    mkc = nc.dram_tensor("mkc", [P, ntiles * CTX], i8, kind="ExternalInput")
    mko = nc.dram_tensor(
        "mko", [P, ntiles * (NEG + 1)], i8, kind="ExternalInput"
    )
    partial = nc.dram_tensor("partial", [P, 1], f32, kind="ExternalOutput")
    scores_out = (
        nc.dram_tensor("scores_out", [P, ntiles * 11], f32, kind="ExternalOutput")
        if dump_scores
        else None
    )

    with tile.TileContext(nc) as tc:
        with (
            tc.tile_pool(name="idxp", bufs=1) as ipool,
            tc.tile_pool(name="gather", bufs=GBUFS) as gpool,
            tc.tile_pool(name="work", bufs=1) as wpool,
            tc.tile_pool(name="stage", bufs=1) as spool,
        ):
          for rep in range(repeats):
            it = ipool.tile([P, ngroups * GW], i16, tag="it")
            nc.sync.dma_start(out=it[:], in_=idx[:])
            itv = it[:].rearrange("p (g c) -> p g c", g=ngroups, c=GW)
            mct = ipool.tile([P, ntiles * CTX], i8, tag="mct")
            nc.sync.dma_start(out=mct[:], in_=mkc[:])
            mcv = mct[:].rearrange(
                "p (g t s) -> p g t s", g=ngroups, t=T, s=CTX
            )
            mot = ipool.tile([P, ntiles * (NEG + 1)], i8, tag="mot")
            nc.sync.dma_start(out=mot[:], in_=mko[:])
            mov = mot[:].rearrange(
                "p (g t s) -> p g t s", g=ngroups, t=T, s=NEG + 1
            )

            scores = spool.tile([P, ntiles * 11], f32, tag="scores")
            scv = scores[:].rearrange(
                "p (g t j) -> p g t j", g=ngroups, t=T, j=11
            )

            for g in range(ngroups):
                ct = gpool.tile([P, CTX * T * ELEM], f16, tag="ct")
                nc.gpsimd.dma_gather(
                    ct[:].rearrange("p (c d) -> p c d", c=CTX * T, d=ELEM),
                    tin[:],
                    itv[:, g, 0:CW],
                    CTX * T * P,
                    CTX * T * P,
                    ELEM,
                    single_packet=False,
                    queue_num=g % QUEUES,
                )
                ot = gpool.tile([P, (NEG + 1) * T * ELEM], f16, tag="ot")
                nc.gpsimd.dma_gather(
                    ot[:].rearrange("p (c d) -> p c d", c=(NEG + 1) * T, d=ELEM),
                    tout[:],
                    itv[:, g, CW:GW],
                    (NEG + 1) * T * P,
                    (NEG + 1) * T * P,
                    ELEM,
                    single_packet=False,
                    queue_num=(g + 2) % QUEUES,
                )
                cv = ct[:].rearrange(
                    "p (t s d) -> p t s d", t=T, s=CTX, d=ELEM
                )
                ov = ot[:].rearrange(
                    "p (t s d) -> p t s d", t=T, s=NEG + 1, d=ELEM
                )

                # ctx rows: in-place 2-way select (block[0:50] <- hi
                # half where mask), then 10 -> 1 tree sum
                mcb = mcv[:, g].unsqueeze(3).broadcast_to((P, T, CTX, DIM))
                nc.vector.copy_predicated(
                    out=cv[:, :, :, 0:DIM],
                    mask=mcb,
                    data=cv[:, :, :, 64 : 64 + DIM],
                )
                s5 = wpool.tile([P, T * 5 * DIM], f16, tag="s5")
                s5v = s5[:].rearrange("p (t s d) -> p t s d", t=T, s=5, d=DIM)
                nc.vector.tensor_add(
                    out=s5v,
                    in0=cv[:, :, 0:5, 0:DIM],
                    in1=cv[:, :, 5:10, 0:DIM],
                )
                s2 = wpool.tile([P, T * 2 * DIM], f16, tag="s2")
                s2v = s2[:].rearrange("p (t s d) -> p t s d", t=T, s=2, d=DIM)
                nc.vector.tensor_add(
                    out=s2v, in0=s5v[:, :, 0:2], in1=s5v[:, :, 2:4]
                )
                s1 = wpool.tile([P, T * 1 * DIM], f16, tag="s1")
                s1v = s1[:].rearrange("p (t s d) -> p t s d", t=T, s=1, d=DIM)
                nc.vector.tensor_add(
                    out=s1v, in0=s2v[:, :, 0:1], in1=s2v[:, :, 1:2]
                )
                ctx = wpool.tile([P, T * 1 * DIM], f16, tag="ctx")
                ctxv = ctx[:].rearrange("p (t s d) -> p t s d", t=T, s=1, d=DIM)
                nc.vector.tensor_add(out=ctxv, in0=s1v, in1=s5v[:, :, 4:5])

                # pos/neg rows: in-place select, then dot with ctx
                mob = mov[:, g].unsqueeze(3).broadcast_to((P, T, NEG + 1, DIM))
                nc.vector.copy_predicated(
                    out=ov[:, :, :, 0:DIM],
                    mask=mob,
                    data=ov[:, :, :, 64 : 64 + DIM],
                )
                so = wpool.tile([P, T * (NEG + 1) * DIM], f16, tag="so")
                sov = so[:].rearrange(
                    "p (t s d) -> p t s d", t=T, s=NEG + 1, d=DIM
                )
                ctxb = ctxv.broadcast_to((P, T, NEG + 1, DIM))
                nc.vector.tensor_mul(
                    out=sov, in0=ov[:, :, :, 0:DIM], in1=ctxb
                )
                nc.vector.tensor_reduce(
                    out=scv[:, g, :, :],
                    in_=sov,
                    axis=mybir.AxisListType.X,
                    op=mybir.AluOpType.add,
                    negate=False,
                )

            acc = spool.tile([P, 1], f32, tag="acc")
            if dump_scores:
                nc.sync.dma_start(out=scores_out[:], in_=scores[:])
            sall = scores[:].rearrange("p (t j) -> p t j", t=ntiles, j=11)
            # -log sig(pos_s) = softplus(-pos_s), -log sig(-neg_s) =
            # softplus(neg_s); softplus(x) = Ln(1 + Exp(x)) keeps both
            # activations in the natural_log_exp table set (one load).
            nc.scalar.activation(
                out=sall[:, :, 0:1],
                in_=sall[:, :, 0:1],
                func=mybir.ActivationFunctionType.Exp,
                scale=-0.1,
            )
            nc.scalar.activation(
                out=sall[:, :, 1:11],
                in_=sall[:, :, 1:11],
                func=mybir.ActivationFunctionType.Exp,
                scale=0.1,
            )
            nc.scalar.activation(
                out=scores[:],
                in_=scores[:],
                func=mybir.ActivationFunctionType.Ln,
                bias=1.0,
                accum_out=acc[:],
            )
            nc.sync.dma_start(out=partial[:], in_=acc[:])

    nc.compile()
    return nc


def _wrap16(flat: np.ndarray) -> np.ndarray:
    """idx_list[i] -> [i%16, i//16], replicated to 128 partitions."""
    w = flat.reshape(-1, 16).T
    return np.tile(w, (8, 1))


def _prep_inputs(context_idxs, pos_target, neg_samples, in_embed_W, out_embed_W):
    ci = np.asarray(context_idxs, dtype=np.int64)  # [B, 10]
    po = np.concatenate(
        [
            np.asarray(pos_target, dtype=np.int64)[:, None],
            np.asarray(neg_samples, dtype=np.int64),
        ],
        axis=1,
    )  # [B, 11]

    def pack(w):
        t = np.zeros((VOCAB, 64), dtype=np.float16)
        t[:, :DIM] = np.asarray(w).astype(np.float16)
        return t.reshape(NBLK, ELEM)

    tin = pack(in_embed_W)
    tout = pack(out_embed_W)

    ngroups = NTILES // T
    in_maps = []
    for c in range(NCORES):
        cic = ci[c * BC : (c + 1) * BC]
        poc = po[c * BC : (c + 1) * BC]
        bc = (cic >> 1).astype(np.int16).reshape(ngroups, T, P, CTX)
        bo = (poc >> 1).astype(np.int16).reshape(ngroups, T, P, NEG + 1)
        cols = []
        for g in range(ngroups):
            cols.append(_wrap16(bc[g].transpose(0, 2, 1).reshape(-1)))
            cols.append(_wrap16(bo[g].transpose(0, 2, 1).reshape(-1)))
        idx_c = np.concatenate(cols, axis=1)  # [128, ngroups*GW]
        mkc = (
            (cic & 1)
            .astype(np.int8)
            .reshape(NTILES, P, CTX)
            .transpose(1, 0, 2)
            .reshape(P, NTILES * CTX)
            .copy()
        )
        mko = (
            (poc & 1)
            .astype(np.int8)
            .reshape(NTILES, P, NEG + 1)
            .transpose(1, 0, 2)
            .reshape(P, NTILES * (NEG + 1))
            .copy()
        )
        in_maps.append(
            {"tin": tin, "tout": tout, "idx": idx_c, "mkc": mkc, "mko": mko}
        )
    return in_maps


def kernel(context_idxs, pos_target, neg_samples, in_embed_W, out_embed_W):
    in_maps = _prep_inputs(
        context_idxs, pos_target, neg_samples, in_embed_W, out_embed_W
    )
    nc = build_nc()
    res = bass_utils.run_bass_kernel_spmd(nc, in_maps, core_ids=list(range(NCORES)))
    # partials are sums of softplus terms = -(log-sigmoid sums), so the
    # loss is +total/B
    total = sum(float(r["partial"].sum()) for r in res.results)
    return np.float32(total / B)


# revision 14
# speedup vs baseline: 1.9121x; 1.9121x over previous
"""CBOW negative-sampling loss kernel for Trainium2 (8 NeuronCores, SPMD).

Per batch element b: gather 21 rows of 50 floats (10 ctx rows from in_embed,
1 pos + 10 neg from out_embed), context sum, 11 dot products, log-sigmoids,
global mean.

v2: bulk gathers via the extended-ISA `dma_gather` (InstDMAGatherAnt)
instead of per-row indirect_dma_start.  One gather instruction fetches
thousands of random table rows (one SDMA descriptor pair per row), so the
~1.8us/op SWDGE fixed cost of the v1 kernel (2688 ops/core -> 4.8ms)
amortizes away.  HW-measured sustained cost is ~2.2-2.8ns per gathered
row at 4 SWDGE queues (the Q7 descriptor-generation floor; HBM-locality
experiments move it <10%), so the 344k rows/core run in ~720us steady
state, ~230-400us under the R=9 pairwise timing that hides pipeline
ramp under dispatch overhead.

dma_gather facts (HW-verified):
- int16 indices, gathered element must be a multiple of 256B.  VOCAB=50000
  rows don't fit int16, so the table is stored fp16 with rows padded to
  128B and viewed as [25000, 256B] blocks: block = v>>1 (max 24999), the
  wanted row is the (v&1) half.  The half-select is one in-place DVE
  copy_predicated (int8 mask, host-uploaded) per gather tile.
- single_packet=True caps a call at 64 descs/engine (1024 idxs); pass
  single_packet=False for larger calls (single_packet makes no measurable
  speed difference either way).
- Index list for a call is wrapped into 16 partitions ([16, n/16],
  idx_list[i] at [i%16, i//16]) and replicated 8x to 128 partitions
  (queue q's Q7 pair reads the replica at partitions 32q..32q+31).
- Gathered element i lands at partition i%128, free column i//128.
- 4 SWDGE queues = 4 independent Q7 desc-gen core pairs; queue count is
  the dominant throughput lever (2 queues is ~2.9x slower end to end).
- Call size barely matters (2.7-3.0 ns/row from 1k to 10k idxs/call);
  T=4 tiles/call with a 6-deep gather pool won the sweep.

Per group of T=4 tiles (512 batch elems) two gathers fetch 10*T*128 ctx
blocks from tin and 11*T*128 pos/neg blocks from tout as
[128, slots, 128f16] tiles, with ctx/out calls of group g pinned to
queues g%4 and (g+2)%4 so all four queues see equal work.  DVE then does
the predicated half-select in place, a 10->1 context tree sum, products
against the broadcast ctx vector, and a 50->1 reduce into fp32 scores
(~10.4k elems/partition/group, fully hidden under the gather pipeline).
Tail as v1: -log sig(+-s) == softplus(-+s) == Ln(1+Exp(-+0.1*s)) (the
0.1 folds the /10 context mean), Ln's accum_out yields per-partition
loss sums.  Host: loss = +(sum of partials) / B.  fp16 table keeps the
end-to-end rel err at ~1e-6 (the v1 fp8 table was ~5e-4).
"""

import sys

import numpy as np

if "/opt/trn_rl_repo" not in sys.path:
    sys.path.insert(0, "/opt/trn_rl_repo")

from concourse import bass, mybir  # noqa: E402
from concourse import bass_utils  # noqa: E402
from concourse import tile  # noqa: E402
from concourse.bacc import Bacc  # noqa: E402

VOCAB = 50000
DIM = 50
B = 131072
CTX = 10
NEG = 10

NCORES = 8
P = 128
BC = B // NCORES  # 16384
NTILES = BC // P  # 128
NBLK = VOCAB // 2  # 25000 gather blocks per table
ELEM = 128  # f16 per gather block (256B)
T = 4  # tiles per gather group
QUEUES = 4  # SWDGE queues (parallel Q7 desc-gen core pairs)
GBUFS = 6  # gather-pool double-buffering depth
SCRATCH = 16384  # SWDGE descriptor-ring carveout bytes per partition
CW = CTX * T * P // 16  # idx cols per ctx call (640)
OW = (NEG + 1) * T * P // 16  # idx cols per out call (704)
GW = CW + OW  # idx cols per group (1344)

f16 = mybir.dt.float16
f32 = mybir.dt.float32
i16 = mybir.dt.int16
i8 = mybir.dt.int8


def build_nc(ntiles: int = NTILES, repeats: int = 1, dump_scores: bool = False):
    nc = Bacc(
        None,
        target_bir_lowering=False,
        num_swdge_queues=QUEUES,
        dynamic_dma_scratch_size=SCRATCH,
    )
    one_t = nc.alloc_sbuf_tensor("const-one", [P, 1], f32)
    nc.gpsimd.memset(one_t.ap(), 1.0)
    nc.const_aps.aps[(f32, 1.0)] = one_t.ap()
    nc.all_engine_barrier()

    tin = nc.dram_tensor("tin", [NBLK, ELEM], f16, kind="ExternalInput")
    tout = nc.dram_tensor("tout", [NBLK, ELEM], f16, kind="ExternalInput")
    ngroups = ntiles // T
    assert ngroups * T == ntiles
    idx = nc.dram_tensor("idx", [P, ngroups * GW], i16, kind="ExternalInput")
    mkc = nc.dram_tensor("mkc", [P, ntiles * CTX], i8, kind="ExternalInput")
    mko = nc.dram_tensor(
        "mko", [P, ntiles * (NEG + 1)], i8, kind="ExternalInput"
    )
    partial = nc.dram_tensor("partial", [P, 1], f32, kind="ExternalOutput")
    scores_out = (
        nc.dram_tensor("scores_out", [P, ntiles * 11], f32, kind="ExternalOutput")
        if dump_scores
        else None
    )

    with tile.TileContext(nc) as tc:
        with (
            tc.tile_pool(name="idxp", bufs=1) as ipool,
            tc.tile_pool(name="gather", bufs=GBUFS) as gpool,
            tc.tile_pool(name="work", bufs=1) as wpool,
            tc.tile_pool(name="stage", bufs=1) as spool,
        ):
          for rep in range(repeats):
            it = ipool.tile([P, ngroups * GW], i16, tag="it")
            nc.sync.dma_start(out=it[:], in_=idx[:])
            itv = it[:].rearrange("p (g c) -> p g c", g=ngroups, c=GW)
            mct = ipool.tile([P, ntiles * CTX], i8, tag="mct")
            nc.sync.dma_start(out=mct[:], in_=mkc[:])
            mcv = mct[:].rearrange(
                "p (g t s) -> p g t s", g=ngroups, t=T, s=CTX
            )
            mot = ipool.tile([P, ntiles * (NEG + 1)], i8, tag="mot")
            nc.sync.dma_start(out=mot[:], in_=mko[:])
            mov = mot[:].rearrange(
                "p (g t s) -> p g t s", g=ngroups, t=T, s=NEG + 1
            )

            scores = spool.tile([P, ntiles * 11], f32, tag="scores")
            scv = scores[:].rearrange(
                "p (g t j) -> p g t j", g=ngroups, t=T, j=11
            )

            for g in range(ngroups):
                ct = gpool.tile([P, CTX * T * ELEM], f16, tag="ct")
                nc.gpsimd.dma_gather(
                    ct[:].rearrange("p (c d) -> p c d", c=CTX * T, d=ELEM),
                    tin[:],
                    itv[:, g, 0:CW],
                    CTX * T * P,
                    CTX * T * P,
                    ELEM,
                    single_packet=False,
                    queue_num=g % QUEUES,
                )
                ot = gpool.tile([P, (NEG + 1) * T * ELEM], f16, tag="ot")
                nc.gpsimd.dma_gather(
                    ot[:].rearrange("p (c d) -> p c d", c=(NEG + 1) * T, d=ELEM),
                    tout[:],
                    itv[:, g, CW:GW],
                    (NEG + 1) * T * P,
                    (NEG + 1) * T * P,
                    ELEM,
                    single_packet=False,
                    queue_num=(g + 2) % QUEUES,
                )
                cv = ct[:].rearrange(
                    "p (t s d) -> p t s d", t=T, s=CTX, d=ELEM
                )
                ov = ot[:].rearrange(
                    "p (t s d) -> p t s d", t=T, s=NEG + 1, d=ELEM
                )

                # ctx rows: in-place 2-way select (block[0:50] <- hi
                # half where mask), then 10 -> 1 tree sum
                mcb = mcv[:, g].unsqueeze(3).broadcast_to((P, T, CTX, DIM))
                nc.vector.copy_predicated(
                    out=cv[:, :, :, 0:DIM],
                    mask=mcb,
                    data=cv[:, :, :, 64 : 64 + DIM],
                )
                s5 = wpool.tile([P, T * 5 * DIM], f16, tag="s5")
                s5v = s5[:].rearrange("p (t s d) -> p t s d", t=T, s=5, d=DIM)
                nc.vector.tensor_add(
                    out=s5v,
                    in0=cv[:, :, 0:5, 0:DIM],
                    in1=cv[:, :, 5:10, 0:DIM],
                )
                s2 = wpool.tile([P, T * 2 * DIM], f16, tag="s2")
                s2v = s2[:].rearrange("p (t s d) -> p t s d", t=T, s=2, d=DIM)
                nc.vector.tensor_add(
                    out=s2v, in0=s5v[:, :, 0:2], in1=s5v[:, :, 2:4]
                )
                s1 = wpool.tile([P, T * 1 * DIM], f16, tag="s1")
                s1v = s1[:].rearrange("p (t s d) -> p t s d", t=T, s=1, d=DIM)
                nc.vector.tensor_add(
                    out=s1v, in0=s2v[:, :, 0:1], in1=s2v[:, :, 1:2]
                )
                ctx = wpool.tile([P, T * 1 * DIM], f16, tag="ctx")
                ctxv = ctx[:].rearrange("p (t s d) -> p t s d", t=T, s=1, d=DIM)
                nc.vector.tensor_add(out=ctxv, in0=s1v, in1=s5v[:, :, 4:5])

                # pos/neg rows: in-place select, then dot with ctx
                mob = mov[:, g].unsqueeze(3).broadcast_to((P, T, NEG + 1, DIM))
                nc.vector.copy_predicated(
                    out=ov[:, :, :, 0:DIM],
                    mask=mob,
                    data=ov[:, :, :, 64 : 64 + DIM],
                )
                so = wpool.tile([P, T * (NEG + 1) * DIM], f16, tag="so")
                sov = so[:].rearrange(
                    "p (t s d) -> p t s d", t=T, s=NEG + 1, d=DIM
                )
                ctxb = ctxv.broadcast_to((P, T, NEG + 1, DIM))
                nc.vector.tensor_mul(
                    out=sov, in0=ov[:, :, :, 0:DIM], in1=ctxb
                )
                nc.vector.tensor_reduce(
                    out=scv[:, g, :, :],
                    in_=sov,
                    axis=mybir.AxisListType.X,
                    op=mybir.AluOpType.add,
                    negate=False,
                )

            acc = spool.tile([P, 1], f32, tag="acc")
            if dump_scores:
                nc.sync.dma_start(out=scores_out[:], in_=scores[:])
            sall = scores[:].rearrange("p (t j) -> p t j", t=ntiles, j=11)
            # -log sig(pos_s) = softplus(-pos_s), -log sig(-neg_s) =
            # softplus(neg_s); softplus(x) = Ln(1 + Exp(x)) keeps both
            # activations in the natural_log_exp table set (one load).
            nc.scalar.activation(
                out=sall[:, :, 0:1],
                in_=sall[:, :, 0:1],
                func=mybir.ActivationFunctionType.Exp,
                scale=-0.1,
            )
            nc.scalar.activation(
                out=sall[:, :, 1:11],
                in_=sall[:, :, 1:11],
                func=mybir.ActivationFunctionType.Exp,
                scale=0.1,
            )
            nc.scalar.activation(
                out=scores[:],
                in_=scores[:],
                func=mybir.ActivationFunctionType.Ln,
                bias=1.0,
                accum_out=acc[:],
            )
            nc.sync.dma_start(out=partial[:], in_=acc[:])

    nc.compile()
    return nc


def _wrap16(flat: np.ndarray) -> np.ndarray:
    """idx_list[i] -> [i%16, i//16], replicated to 128 partitions."""
    w = flat.reshape(-1, 16).T
    return np.tile(w, (8, 1))


def _prep_inputs(context_idxs, pos_target, neg_samples, in_embed_W, out_embed_W):
    ci = np.asarray(context_idxs, dtype=np.int64)  # [B, 10]
    po = np.concatenate(
        [
            np.asarray(pos_target, dtype=np.int64)[:, None],
            np.asarray(neg_samples, dtype=np.int64),
        ],
        axis=1,
    )  # [B, 11]

    def pack(w):
        t = np.zeros((VOCAB, 64), dtype=np.float16)
        t[:, :DIM] = np.asarray(w).astype(np.float16)
        return t.reshape(NBLK, ELEM)

    tin = pack(in_embed_W)
    tout = pack(out_embed_W)

    ngroups = NTILES // T
    in_maps = []
    for c in range(NCORES):
        cic = ci[c * BC : (c + 1) * BC]
        poc = po[c * BC : (c + 1) * BC]
        bc = (cic >> 1).astype(np.int16).reshape(ngroups, T, P, CTX)
        bo = (poc >> 1).astype(np.int16).reshape(ngroups, T, P, NEG + 1)
        cols = []
        for g in range(ngroups):
            cols.append(_wrap16(bc[g].transpose(0, 2, 1).reshape(-1)))
            cols.append(_wrap16(bo[g].transpose(0, 2, 1).reshape(-1)))
        idx_c = np.concatenate(cols, axis=1)  # [128, ngroups*GW]
        mkc = (
            (cic & 1)
            .astype(np.int8)
            .reshape(NTILES, P, CTX)
            .transpose(1, 0, 2)
            .reshape(P, NTILES * CTX)
            .copy()
        )
        mko = (
            (poc & 1)
            .astype(np.int8)
            .reshape(NTILES, P, NEG + 1)
            .transpose(1, 0, 2)
            .reshape(P, NTILES * (NEG + 1))
            .copy()
        )
        in_maps.append(
            {"tin": tin, "tout": tout, "idx": idx_c, "mkc": mkc, "mko": mko}
        )
    return in_maps


def kernel(context_idxs, pos_target, neg_samples, in_embed_W, out_embed_W):
    in_maps = _prep_inputs(
        context_idxs, pos_target, neg_samples, in_embed_W, out_embed_W
    )
    nc = build_nc()
    try:
        res = bass_utils.run_bass_kernel_spmd(
            nc, in_maps, core_ids=list(range(NCORES))
        )
    except Exception:
        # one retry: the axon-tunneled devices occasionally report a
        # transient NRT error under load; a fresh execute recovers
        res = bass_utils.run_bass_kernel_spmd(
            nc, in_maps, core_ids=list(range(NCORES))
        )
    # partials are sums of softplus terms = -(log-sigmoid sums), so the
    # loss is +total/B
    total = sum(float(r["partial"].sum()) for r in res.results)
    return np.float32(total / B)
